# revision 19
# baseline (speedup 1.0000x reference)
"""Trainium2 Bass kernel: nn_BlockMLP_MixerBlock (2-layer butterfly block-MLP mixer).

Math (per batch row pair; BS=16384, D=2048, BD=64, NB=32, H=128):
  L0: per block n: o = gelu(y @ W1_0[n]) @ W2_0[n]   (biases are zeros by spec)
  P1 butterfly: element (b=2q+r, block n, pos j=32u+v) -> (b'=2q+u, block v, pos 32r+n)
  L1: same block-MLP with W*_1;  P2 = same involution.

v5: all-bf16 PE; software-pipelined chunk emission (input stage of chunk i
interleaves with the body of chunk i-1); copies are blits or low-dim APs; the
matmul-rhs single-free-dim rule is satisfied with single *strided* dims
(z1 read at stride 32, out-transpose reads at stride 2).

Batch labels within a chunk of 512 rows (256 pairs): pair q_local =
128*pt + 64*q0 + ph (pt = load tile, q0 = sbm partition hi-bit, ph = low 6),
parity r.  qqb := pt.  Layouts per chunk (bf16 except outsb):
  xT    [128 p=f%128,  free (t 16, q0 2, pt 2, r 2, ph 64)]
  hsb   [128 p=H,      free (s 2, q0 2, pt 2, r 2, ph 64)]     gelu blit
  o0sb  [128 p=(q0,32u+v), free (ph 64, pt 2, r 2, t 16, s 2)]
  z1sb  [128 p=(qqb,32r+n), free (g 8, k 8, q0 2, u 2, v 32)]  blit of psM
  o1sb  [128 p=(qqb,32r'+n'), free (v 32, gk 64, q0 2, u 2)]   blit of o1ps
  outsb [128 p=(2ph+q0), free (qqb 2, r' 2, n' 32, u 2, v 32)] f32
"""
import numpy as np

BS, D, BD, NB, H = 16384, 2048, 64, 32, 128
NCORES = 8
BCORE = BS // NCORES     # 2048
CB = 512                 # chunk rows
NCH = BCORE // CB        # 4

_module_cache = {}


def build(act="gelu", stages=5):
    import concourse.mybir as mybir
    from concourse import bacc
    from concourse.tile import TileContext
    from concourse.masks import make_identity

    f32 = mybir.dt.float32
    bf16 = mybir.dt.bfloat16
    AF = mybir.ActivationFunctionType
    act_fn = AF.Gelu if act == "gelu" else AF.Copy

    nc = bacc.Bacc("TRN2", target_bir_lowering=False)
    x = nc.dram_tensor("x", (BCORE, D), bf16, kind="ExternalInput")
    W1d = [nc.dram_tensor("W1_0", (NB, BD, H), bf16, kind="ExternalInput"),
           nc.dram_tensor("W1_1", (NB, BD, H), bf16, kind="ExternalInput")]
    W2d = [nc.dram_tensor("W2_0", (NB, H, BD), bf16, kind="ExternalInput"),
           nc.dram_tensor("W2_1", (NB, H, BD), bf16, kind="ExternalInput")]
    out = nc.dram_tensor("out", (BCORE, D), f32, kind="ExternalOutput")

    with TileContext(nc) as tc:
        with tc.tile_pool(name="wp", bufs=1) as wp, \
             tc.tile_pool(name="sbp", bufs=4) as sbp, \
             tc.tile_pool(name="xtp", bufs=2) as xtp, \
             tc.tile_pool(name="o0p", bufs=2) as o0p, \
             tc.tile_pool(name="z1p", bufs=2) as z1p, \
             tc.tile_pool(name="o1p", bufs=1) as o1p, \
             tc.tile_pool(name="outp", bufs=1) as outp, \
             tc.tile_pool(name="wk", bufs=3) as wk, \
             tc.tile_pool(name="pst", bufs=2, space="PSUM") as pst, \
             tc.tile_pool(name="pss", bufs=2, space="PSUM") as pss, \
             tc.tile_pool(name="psh", bufs=2, space="PSUM") as psh:

            # ---------------- weights (staged once; scalar queue) ----------
            ident = wp.tile([128, 128], bf16, name="ident", tag="ident")
            make_identity(nc, ident)

            # L0 MM1: lhsT for block n=2t+s at partitions [64s,64s+64)
            w1l0 = wp.tile([128, 16 * 128], bf16, name="w1l0", tag="w1l0")
            w1r0 = W1d[0].rearrange("(t s) c m -> s c t m", s=2)
            for s in range(2):
                nc.scalar.dma_start(out=w1l0[64 * s:64 * s + 64, :], in_=w1r0[s])
            # L1 MM1: W1_1 duplicated in both partition halves; used with
            # tile_position=(64*qqb, 0) and K=64 rhs slices.
            w1l1 = wp.tile([128, 32 * 128], bf16, name="w1l1", tag="w1l1")
            w1r1 = W1d[1].rearrange("v c m -> c v m")
            for h2 in range(2):
                nc.scalar.dma_start(out=w1l1[64 * h2:64 * h2 + 64, :], in_=w1r1)
            # MM2 (both layers): interleaved [Z | W2_l | Z | ... | Z]
            # A(n) = [:, 128n+64:+128] = [W2_n | Z], B(n) = [:, 128n:+128]
            w2l = []
            for l in range(2):
                w2t = wp.tile([128, 64 * (2 * NB + 1)], bf16,
                              name=f"w2l{l}", tag=f"w2l{l}")
                zv = w2t.rearrange("p (m c) -> p m c", c=64)[:, ::2]
                nc.gpsimd.memset(zv, 0.0)
                dst = w2t[:, 0:4096].rearrange("p (n c) -> p n c", c=128)[:, :, 64:128]
                nc.scalar.dma_start(out=dst,
                                    in_=W2d[l].rearrange("n m j -> m n j"))
                w2l.append(w2t)

            xv = x.rearrange("(q r) f -> q r f", r=2)   # q global pair, r parity

            def emit_loads(ch):
                sbms = {}
                for r in range(2):
                    for pt in range(2):
                        sbm = sbp.tile([128, D], bf16, name=f"sbm{r}{pt}",
                                       tag="sbm")
                        q0g = ch * 256 + 128 * pt
                        nc.sync.dma_start(out=sbm, in_=xv[q0g:q0g + 128, r, :])
                        sbms[(r, pt)] = sbm
                return sbms

            IN_GROUPS = [(g8, r, pt) for g8 in range(2)
                         for r in range(2) for pt in range(2)]

            def alloc_xT():
                # xT col (within t) = q0*256 + pt*128 + r*64 + ph
                return xtp.tile([128, 8192], bf16, name="xT", tag="xT")

            def emit_in_group(xT, sbms, gi):
                g8, r, pt = IN_GROUPS[gi]
                xTr = xT.rearrange("p (t q0x pt2 r2 ph) -> r2 pt2 p t q0x ph",
                                   t=16, q0x=2, pt2=2, r2=2, ph=64)
                sbm = sbms[(r, pt)]
                psT = pst.tile([128, 1024], bf16, name="psT", tag="tr")
                for k in range(8):
                    ft = 8 * g8 + k
                    nc.tensor.transpose(
                        out=psT[:, 128 * k:128 * k + 128],
                        in_=sbm[:, 128 * ft:128 * ft + 128],
                        identity=ident)
                nc.vector.tensor_copy(
                    out=xTr[r, pt][:, 8 * g8:8 * g8 + 8],
                    in_=psT.rearrange("p (k q0x ph) -> p k q0x ph",
                                      k=8, q0x=2, ph=64))

            def emit_in_transposes(sbms):
                xT = alloc_xT()
                for gi in range(8):
                    emit_in_group(xT, sbms, gi)
                return xT

            def emit_l0(ch, xT, nxt=None):
                # ---------------- layer 0 ----------------
                o0sb = o0p.tile([128, 8192], bf16, name="o0sb", tag="o0")
                o0m = o0sb.rearrange("p (ph ptr nt s) -> nt p ptr ph s",
                                     ph=64, ptr=4, nt=16, s=2)

                def l0_mm1(t):
                    hps = psh.tile([128, 1024], f32, name="hps", tag="h")
                    for s in range(2):
                        nc.tensor.matmul(
                            hps[:, 512 * s:512 * s + 512],
                            w1l0[64 * s:64 * s + 64, 128 * t:128 * t + 128],
                            xT[64 * s:64 * s + 64, 512 * t:512 * t + 512],
                            start=True, stop=True, tile_position=(64 * s, 0))
                    hsb = wk.tile([128, 1024], bf16, name="hsb", tag="hsb")
                    nc.scalar.activation(hsb, hps, act_fn)
                    return hsb

                # MM1(t+1) is emitted before MM2(t): the in-order PE queue
                # then streams matmuls while ACT computes gelu(t).
                hq = [l0_mm1(0), l0_mm1(1)]
                for t in range(16):
                    if t + 2 < 16:
                        hq.append(l0_mm1(t + 2))
                    hsb = hq.pop(0)
                    ops = pss.tile([128, 512], f32, name="ops", tag="mm")
                    for s in range(2):
                        n = 2 * t + s
                        # zero-pad accumulate: out partitions (q0, j=32u+v)
                        nc.tensor.matmul(
                            ops[:, 256 * s:256 * s + 256],
                            w2l[0][:, 128 * n + 64:128 * n + 192],
                            hsb[:, 512 * s:512 * s + 256],
                            start=True, stop=False)
                        nc.tensor.matmul(
                            ops[:, 256 * s:256 * s + 256],
                            w2l[0][:, 128 * n:128 * n + 128],
                            hsb[:, 512 * s + 256:512 * s + 512],
                            start=False, stop=True)
                    # one 3D copy per t: dims (ptr, ph, s); dst s-stride 1
                    opr = ops.rearrange("p (s ptr ph) -> p ptr ph s",
                                        s=2, ptr=4, ph=64)
                    nc.vector.tensor_copy(out=o0m[t], in_=opr)
                    # interleave next chunk's input transposes into L0
                    if nxt is not None and t % 2 == 1:
                        emit_in_group(nxt[0], nxt[1], t // 2)

                if stages <= 2:
                    dmp = out.bitcast(bf16).rearrange(
                        "(c p g) f -> c p (g f)", c=NCH, p=128, g=4)[ch]
                    nc.scalar.dma_start(out=dmp[:, :8192], in_=o0sb)
                    return None
                # -------- mid transpose (P1): one XBAR DMA-transpose --------
                # out[p][ph][c] = o0sb[c][128*ph+p]: partitions (qqb, rn),
                # per-ph block c = (q0, u, v) — exactly z1sb's layout.
                z1sb = z1p.tile([128, 8192], bf16, name="z1sb", tag="z1")
                nc.sync.dma_start_transpose(
                    out=z1sb.rearrange("p (ph c) -> p ph c", ph=64, c=128),
                    in_=o0sb)
                return z1sb

            def emit_l1(ch, z1sb):
                if stages <= 3:
                    dmp = out.bitcast(bf16).rearrange(
                        "(c p g) f -> c p (g f)", c=NCH, p=128, g=4)[ch]
                    nc.scalar.dma_start(out=dmp[:, :8192], in_=z1sb)
                    return
                # ---------------- layer 1 ----------------
                # z1 col = 32*j + v with j = (g k q0 u): rhs per v is one
                # strided free dim (stride 32).  h1/o1 cols iterate j.
                z1v = z1sb.rearrange("p (j v) -> v p j", j=256, v=32)
                o1sb = o1p.tile([128, 8192], bf16, name="o1sb", tag="o1")

                def l1_mm1(G):
                    # h1ps cols = (qqb, w, j): tile_position pair must hit
                    # different PSUM banks (concurrent quadrant matmuls).
                    h1ps = psh.tile([128, 1024], f32, name="h1ps", tag="h")
                    for w in range(2):
                        v = 2 * G + w
                        for qqb in range(2):
                            nc.tensor.matmul(
                                h1ps[:, 512 * qqb + 256 * w:512 * qqb + 256 * w + 256],
                                w1l1[64 * qqb:64 * qqb + 64, 128 * v:128 * v + 128],
                                z1v[v][64 * qqb:64 * qqb + 64],
                                start=True, stop=True,
                                tile_position=(64 * qqb, 0))
                    h1sb = wk.tile([128, 1024], bf16, name="h1sb", tag="h1sb")
                    nc.scalar.activation(h1sb, h1ps, act_fn)
                    return h1sb

                # out-transpose plumbing (interleaved into the L1 loop:
                # group (u, G4) unlocks after o1 blits 4*G4..4*G4+3)
                o1r = o1sb.rearrange("p (v j2 u) -> v u p j2",
                                     v=32, j2=128, u=2)
                outsb = outp.tile([128, 8192], f32, name="outsb", tag="outsb")
                outr = outsb.rearrange(
                    "p (qqb jhi jlo uu gg k) -> gg uu qqb p k jhi jlo",
                    qqb=2, jhi=2, jlo=32, uu=2, gg=4, k=8)

                def emit_outT(G4):
                    for u in range(2):
                        psO = pst.tile([128, 1024], bf16, name="psO", tag="tr")
                        for k in range(8):
                            v = 8 * G4 + k
                            nc.tensor.transpose(
                                out=psO[:, 128 * k:128 * k + 128],
                                in_=o1r[v, u],
                                identity=ident)
                        psr = psO.rearrange(
                            "p (k qqb jhi jlo) -> qqb p k jhi jlo",
                            k=8, qqb=2, jhi=2, jlo=32)
                        nc.vector.tensor_copy(out=outr[G4, u, 0], in_=psr[0])
                        nc.scalar.activation(outr[G4, u, 1], psr[1], AF.Copy)

                h1q = [l1_mm1(0), l1_mm1(1)]
                for G in range(16):  # 2 blocks per group
                    if G + 2 < 16:
                        h1q.append(l1_mm1(G + 2))
                    h1sb = h1q.pop(0)
                    o1ps = pss.tile([128, 512], f32, name="o1ps", tag="mm")
                    for w in range(2):
                        v = 2 * G + w
                        # zero-pad accumulate: out partitions (qqb, 32r'+n')
                        nc.tensor.matmul(
                            o1ps[:, 256 * w:256 * w + 256],
                            w2l[1][:, 128 * v + 64:128 * v + 192],
                            h1sb[:, 256 * w:256 * w + 256],
                            start=True, stop=False)
                        nc.tensor.matmul(
                            o1ps[:, 256 * w:256 * w + 256],
                            w2l[1][:, 128 * v:128 * v + 128],
                            h1sb[:, 512 + 256 * w:512 + 256 * w + 256],
                            start=False, stop=True)
                    nc.vector.tensor_copy(
                        out=o1sb[:, 512 * G:512 * G + 512], in_=o1ps)
                    if stages > 4 and G % 4 == 3:
                        emit_outT(G // 4)

                if stages <= 4:
                    dmp = out.bitcast(bf16).rearrange(
                        "(c p g) f -> c p (g f)", c=NCH, p=128, g=4)[ch]
                    nc.scalar.dma_start(out=dmp[:, :8192], in_=o1sb)
                    return
                # ---------------- store (scalar queue) ----------------
                # DRAM row = ch*512 + qqb*256 + q0*128 + ph*2 + r_out
                ov = out.rearrange("(c qx q0x pp jhi) f -> c qx pp q0x (jhi f)",
                                   c=NCH, qx=2, q0x=2, pp=64, jhi=2)
                nc.scalar.dma_start(out=ov[ch, 0], in_=outsb[:, 0:4096])
                eng_st = nc.sync if ch == NCH - 1 else nc.scalar
                eng_st.dma_start(out=ov[ch, 1], in_=outsb[:, 4096:8192])

            # ---------------- software-pipelined emission ----------------
            if stages <= 1:
                sbms = emit_loads(0)
                for i in range(NCH):
                    nxt_sb = emit_loads(i + 1) if i + 1 < NCH else None
                    xT = emit_in_transposes(sbms)
                    dmp = out.bitcast(bf16).rearrange(
                        "(c p g) f -> c p (g f)", c=NCH, p=128, g=4)[i]
                    nc.scalar.dma_start(out=dmp[:, :8192], in_=xT)
                    sbms = nxt_sb
            else:
                sbms0 = emit_loads(0)
                sbms1 = emit_loads(1)
                xT_cur = emit_in_transposes(sbms0)
                z1_prev = None
                for ch in range(NCH):
                    nxt = None
                    if ch + 1 < NCH:
                        nxt = (alloc_xT(), sbms1)
                    z1_cur = emit_l0(ch, xT_cur, nxt)
                    if z1_prev is not None:
                        emit_l1(ch - 1, z1_prev)
                    z1_prev = z1_cur
                    if ch + 2 < NCH:
                        sbms1 = emit_loads(ch + 2)
                    if nxt is not None:
                        xT_cur = nxt[0]
                if z1_prev is not None:
                    emit_l1(NCH - 1, z1_prev)

    nc.compile()
    return nc


def _get_module():
    if "m" not in _module_cache:
        _module_cache["m"] = build(act="gelu")
    return _module_cache["m"]


def kernel(**inputs):
    import ml_dtypes
    from concourse import bass_utils
    nc = _get_module()
    bf = ml_dtypes.bfloat16
    x = np.ascontiguousarray(np.asarray(inputs["x"]).astype(bf))
    names = ["W1_0", "W1_1", "W2_0", "W2_1"]
    wmap = {k: np.ascontiguousarray(np.asarray(inputs[k]).astype(bf))
            for k in names}
    in_maps = []
    for c in range(NCORES):
        m = dict(wmap)
        m["x"] = np.ascontiguousarray(x[c * BCORE:(c + 1) * BCORE])
        in_maps.append(m)
    res = bass_utils.run_bass_kernel_spmd(nc, in_maps, core_ids=list(range(NCORES)))
    return np.concatenate([res.results[c]["out"] for c in range(NCORES)], axis=0)


# revision 22
# speedup vs baseline: 1.0081x; 1.0081x over previous
"""Trainium2 Bass kernel: nn_BlockMLP_MixerBlock (2-layer butterfly block-MLP mixer).

Math (per batch row pair; BS=16384, D=2048, BD=64, NB=32, H=128):
  L0: per block n: o = gelu(y @ W1_0[n]) @ W2_0[n]   (biases are zeros by spec)
  P1 butterfly: element (b=2q+r, block n, pos j=32u+v) -> (b'=2q+u, block v, pos 32r+n)
  L1: same block-MLP with W*_1;  P2 = same involution.

v5: all-bf16 PE; software-pipelined chunk emission (input stage of chunk i
interleaves with the body of chunk i-1); copies are blits or low-dim APs; the
matmul-rhs single-free-dim rule is satisfied with single *strided* dims
(z1 read at stride 32, out-transpose reads at stride 2).

Batch labels within a chunk of 512 rows (256 pairs): pair q_local =
128*pt + 64*q0 + ph (pt = load tile, q0 = sbm partition hi-bit, ph = low 6),
parity r.  qqb := pt.  Layouts per chunk (bf16 except outsb):
  xT    [128 p=f%128,  free (t 16, q0 2, pt 2, r 2, ph 64)]
  hsb   [128 p=H,      free (s 2, q0 2, pt 2, r 2, ph 64)]     gelu blit
  o0sb  [128 p=(q0,32u+v), free (ph 64, pt 2, r 2, t 16, s 2)]
  z1sb  [128 p=(qqb,32r+n), free (g 8, k 8, q0 2, u 2, v 32)]  blit of psM
  o1sb  [128 p=(qqb,32r'+n'), free (v 32, gk 64, q0 2, u 2)]   blit of o1ps
  outsb [128 p=(2ph+q0), free (qqb 2, r' 2, n' 32, u 2, v 32)] f32
"""
import numpy as np

BS, D, BD, NB, H = 16384, 2048, 64, 32, 128
NCORES = 8
BCORE = BS // NCORES     # 2048
CB = 512                 # chunk rows
NCH = BCORE // CB        # 4

_module_cache = {}


def build(act="gelu", stages=5):
    import concourse.mybir as mybir
    from concourse import bacc
    from concourse.tile import TileContext
    from concourse.masks import make_identity

    f32 = mybir.dt.float32
    bf16 = mybir.dt.bfloat16
    AF = mybir.ActivationFunctionType
    act_fn = AF.Gelu if act == "gelu" else AF.Copy

    nc = bacc.Bacc("TRN2", target_bir_lowering=False)
    x = nc.dram_tensor("x", (BCORE, D), bf16, kind="ExternalInput")
    W1d = [nc.dram_tensor("W1_0", (NB, BD, H), bf16, kind="ExternalInput"),
           nc.dram_tensor("W1_1", (NB, BD, H), bf16, kind="ExternalInput")]
    W2d = [nc.dram_tensor("W2_0", (NB, H, BD), bf16, kind="ExternalInput"),
           nc.dram_tensor("W2_1", (NB, H, BD), bf16, kind="ExternalInput")]
    out = nc.dram_tensor("out", (BCORE, D), f32, kind="ExternalOutput")

    with TileContext(nc) as tc:
        with tc.tile_pool(name="wp", bufs=1) as wp, \
             tc.tile_pool(name="sbp", bufs=4) as sbp, \
             tc.tile_pool(name="xtp", bufs=2) as xtp, \
             tc.tile_pool(name="o0p", bufs=2) as o0p, \
             tc.tile_pool(name="z1p", bufs=2) as z1p, \
             tc.tile_pool(name="o1p", bufs=1) as o1p, \
             tc.tile_pool(name="outp", bufs=1) as outp, \
             tc.tile_pool(name="wk", bufs=3) as wk, \
             tc.tile_pool(name="pst", bufs=2, space="PSUM") as pst, \
             tc.tile_pool(name="pss", bufs=2, space="PSUM") as pss, \
             tc.tile_pool(name="psh", bufs=2, space="PSUM") as psh:

            # ---------------- weights (staged once; scalar queue) ----------
            ident = wp.tile([128, 128], bf16, name="ident", tag="ident")
            make_identity(nc, ident)

            # L0 MM1: lhsT for block n=2t+s at partitions [64s,64s+64)
            w1l0 = wp.tile([128, 16 * 128], bf16, name="w1l0", tag="w1l0")
            w1r0 = W1d[0].rearrange("(t s) c m -> s c t m", s=2)
            for s in range(2):
                nc.scalar.dma_start(out=w1l0[64 * s:64 * s + 64, :], in_=w1r0[s])
            # L1 MM1: W1_1 duplicated in both partition halves; used with
            # tile_position=(64*qqb, 0) and K=64 rhs slices.
            w1l1 = wp.tile([128, 32 * 128], bf16, name="w1l1", tag="w1l1")
            w1r1 = W1d[1].rearrange("v c m -> c v m")
            for h2 in range(2):
                nc.scalar.dma_start(out=w1l1[64 * h2:64 * h2 + 64, :], in_=w1r1)
            # MM2 (both layers): interleaved [Z | W2_l | Z | ... | Z]
            # A(n) = [:, 128n+64:+128] = [W2_n | Z], B(n) = [:, 128n:+128]
            w2l = []
            for l in range(2):
                w2t = wp.tile([128, 64 * (2 * NB + 1)], bf16,
                              name=f"w2l{l}", tag=f"w2l{l}")
                zv = w2t.rearrange("p (m c) -> p m c", c=64)[:, ::2]
                nc.gpsimd.memset(zv, 0.0)
                dst = w2t[:, 0:4096].rearrange("p (n c) -> p n c", c=128)[:, :, 64:128]
                nc.scalar.dma_start(out=dst,
                                    in_=W2d[l].rearrange("n m j -> m n j"))
                w2l.append(w2t)

            xv = x.rearrange("(q r) f -> q r f", r=2)   # q global pair, r parity

            def emit_loads(ch):
                sbms = {}
                for r in range(2):
                    for pt in range(2):
                        sbm = sbp.tile([128, D], bf16, name=f"sbm{r}{pt}",
                                       tag="sbm")
                        q0g = ch * 256 + 128 * pt
                        nc.sync.dma_start(out=sbm, in_=xv[q0g:q0g + 128, r, :])
                        sbms[(r, pt)] = sbm
                return sbms

            IN_GROUPS = [(g8, r, pt) for g8 in range(2)
                         for r in range(2) for pt in range(2)]

            def alloc_xT():
                # xT col (within t) = q0*256 + pt*128 + r*64 + ph
                return xtp.tile([128, 8192], bf16, name="xT", tag="xT")

            def emit_in_group(xT, sbms, gi):
                g8, r, pt = IN_GROUPS[gi]
                xTr = xT.rearrange("p (t q0x pt2 r2 ph) -> r2 pt2 p t q0x ph",
                                   t=16, q0x=2, pt2=2, r2=2, ph=64)
                sbm = sbms[(r, pt)]
                psT = pst.tile([128, 1024], bf16, name="psT", tag="tr")
                for k in range(8):
                    ft = 8 * g8 + k
                    nc.tensor.transpose(
                        out=psT[:, 128 * k:128 * k + 128],
                        in_=sbm[:, 128 * ft:128 * ft + 128],
                        identity=ident)
                nc.vector.tensor_copy(
                    out=xTr[r, pt][:, 8 * g8:8 * g8 + 8],
                    in_=psT.rearrange("p (k q0x ph) -> p k q0x ph",
                                      k=8, q0x=2, ph=64))

            def emit_in_transposes(sbms):
                xT = alloc_xT()
                for gi in range(8):
                    emit_in_group(xT, sbms, gi)
                return xT

            def emit_l0(ch, xT, nxt=None, postload=None):
                # ---------------- layer 0 ----------------
                o0sb = o0p.tile([128, 8192], bf16, name="o0sb", tag="o0")
                o0m = o0sb.rearrange("p (ph ptr nt s) -> nt p ptr ph s",
                                     ph=64, ptr=4, nt=16, s=2)

                def l0_mm1(t):
                    hps = psh.tile([128, 1024], f32, name="hps", tag="h")
                    for s in range(2):
                        nc.tensor.matmul(
                            hps[:, 512 * s:512 * s + 512],
                            w1l0[64 * s:64 * s + 64, 128 * t:128 * t + 128],
                            xT[64 * s:64 * s + 64, 512 * t:512 * t + 512],
                            start=True, stop=True, tile_position=(64 * s, 0))
                    hsb = wk.tile([128, 1024], bf16, name="hsb", tag="hsb")
                    nc.scalar.activation(hsb, hps, act_fn)
                    return hsb

                # MM1(t+1) is emitted before MM2(t): the in-order PE queue
                # then streams matmuls while ACT computes gelu(t).
                hq = [l0_mm1(0), l0_mm1(1)]
                for t in range(16):
                    if t + 2 < 16:
                        hq.append(l0_mm1(t + 2))
                    hsb = hq.pop(0)
                    ops = pss.tile([128, 512], f32, name="ops", tag="mm")
                    for s in range(2):
                        n = 2 * t + s
                        # zero-pad accumulate: out partitions (q0, j=32u+v)
                        nc.tensor.matmul(
                            ops[:, 256 * s:256 * s + 256],
                            w2l[0][:, 128 * n + 64:128 * n + 192],
                            hsb[:, 512 * s:512 * s + 256],
                            start=True, stop=False)
                        nc.tensor.matmul(
                            ops[:, 256 * s:256 * s + 256],
                            w2l[0][:, 128 * n:128 * n + 128],
                            hsb[:, 512 * s + 256:512 * s + 512],
                            start=False, stop=True)
                    # one 3D copy per t: dims (ptr, ph, s); dst s-stride 1
                    opr = ops.rearrange("p (s ptr ph) -> p ptr ph s",
                                        s=2, ptr=4, ph=64)
                    nc.vector.tensor_copy(out=o0m[t], in_=opr)
                    # interleave next chunk's input transposes into L0
                    if nxt is not None and t % 2 == 1:
                        emit_in_group(nxt[0], nxt[1], t // 2)

                pl = postload() if postload is not None else None
                if stages <= 2:
                    dmp = out.bitcast(bf16).rearrange(
                        "(c p g) f -> c p (g f)", c=NCH, p=128, g=4)[ch]
                    nc.scalar.dma_start(out=dmp[:, :8192], in_=o0sb)
                    return None, pl
                # -------- mid transpose (P1): one XBAR DMA-transpose --------
                # out[p][ph][c] = o0sb[c][128*ph+p]: partitions (qqb, rn),
                # per-ph block c = (q0, u, v) — exactly z1sb's layout.
                z1sb = z1p.tile([128, 8192], bf16, name="z1sb", tag="z1")
                nc.sync.dma_start_transpose(
                    out=z1sb.rearrange("p (ph c) -> p ph c", ph=64, c=128),
                    in_=o0sb)
                return z1sb, pl

            def emit_l1(ch, z1sb):
                if stages <= 3:
                    dmp = out.bitcast(bf16).rearrange(
                        "(c p g) f -> c p (g f)", c=NCH, p=128, g=4)[ch]
                    nc.scalar.dma_start(out=dmp[:, :8192], in_=z1sb)
                    return
                # ---------------- layer 1 ----------------
                # z1 col = 32*j + v with j = (g k q0 u): rhs per v is one
                # strided free dim (stride 32).  h1/o1 cols iterate j.
                z1v = z1sb.rearrange("p (j v) -> v p j", j=256, v=32)
                o1sb = o1p.tile([128, 8192], bf16, name="o1sb", tag="o1")

                def l1_mm1(G):
                    # h1ps cols = (qqb, w, j): tile_position pair must hit
                    # different PSUM banks (concurrent quadrant matmuls).
                    h1ps = psh.tile([128, 1024], f32, name="h1ps", tag="h")
                    for w in range(2):
                        v = 2 * G + w
                        for qqb in range(2):
                            nc.tensor.matmul(
                                h1ps[:, 512 * qqb + 256 * w:512 * qqb + 256 * w + 256],
                                w1l1[64 * qqb:64 * qqb + 64, 128 * v:128 * v + 128],
                                z1v[v][64 * qqb:64 * qqb + 64],
                                start=True, stop=True,
                                tile_position=(64 * qqb, 0))
                    h1sb = wk.tile([128, 1024], bf16, name="h1sb", tag="h1sb")
                    nc.scalar.activation(h1sb, h1ps, act_fn)
                    return h1sb

                # out-transpose plumbing (interleaved into the L1 loop:
                # group (u, G4) unlocks after o1 blits 4*G4..4*G4+3)
                o1r = o1sb.rearrange("p (v j2 u) -> v u p j2",
                                     v=32, j2=128, u=2)
                outsb = outp.tile([128, 8192], f32, name="outsb", tag="outsb")
                outr = outsb.rearrange(
                    "p (qqb jhi jlo uu gg k) -> gg uu qqb p k jhi jlo",
                    qqb=2, jhi=2, jlo=32, uu=2, gg=4, k=8)

                def emit_outT(G4):
                    for u in range(2):
                        psO = pst.tile([128, 1024], bf16, name="psO", tag="tr")
                        for k in range(8):
                            v = 8 * G4 + k
                            nc.tensor.transpose(
                                out=psO[:, 128 * k:128 * k + 128],
                                in_=o1r[v, u],
                                identity=ident)
                        psr = psO.rearrange(
                            "p (k qqb jhi jlo) -> qqb p k jhi jlo",
                            k=8, qqb=2, jhi=2, jlo=32)
                        nc.vector.tensor_copy(out=outr[G4, u, 0], in_=psr[0])
                        nc.scalar.activation(outr[G4, u, 1], psr[1], AF.Copy)

                h1q = [l1_mm1(0), l1_mm1(1)]
                for G in range(16):  # 2 blocks per group
                    if G + 2 < 16:
                        h1q.append(l1_mm1(G + 2))
                    h1sb = h1q.pop(0)
                    o1ps = pss.tile([128, 512], f32, name="o1ps", tag="mm")
                    for w in range(2):
                        v = 2 * G + w
                        # zero-pad accumulate: out partitions (qqb, 32r'+n')
                        nc.tensor.matmul(
                            o1ps[:, 256 * w:256 * w + 256],
                            w2l[1][:, 128 * v + 64:128 * v + 192],
                            h1sb[:, 256 * w:256 * w + 256],
                            start=True, stop=False)
                        nc.tensor.matmul(
                            o1ps[:, 256 * w:256 * w + 256],
                            w2l[1][:, 128 * v:128 * v + 128],
                            h1sb[:, 512 + 256 * w:512 + 256 * w + 256],
                            start=False, stop=True)
                    nc.vector.tensor_copy(
                        out=o1sb[:, 512 * G:512 * G + 512], in_=o1ps)
                    if stages > 4 and G % 4 == 3:
                        emit_outT(G // 4)

                if stages <= 4:
                    dmp = out.bitcast(bf16).rearrange(
                        "(c p g) f -> c p (g f)", c=NCH, p=128, g=4)[ch]
                    nc.scalar.dma_start(out=dmp[:, :8192], in_=o1sb)
                    return
                # ---------------- store (scalar queue) ----------------
                # DRAM row = ch*512 + qqb*256 + q0*128 + ph*2 + r_out
                ov = out.rearrange("(c qx q0x pp jhi) f -> c qx pp q0x (jhi f)",
                                   c=NCH, qx=2, q0x=2, pp=64, jhi=2)
                nc.scalar.dma_start(out=ov[ch, 0], in_=outsb[:, 0:4096])
                eng_st = nc.sync if ch == NCH - 1 else nc.scalar
                eng_st.dma_start(out=ov[ch, 1], in_=outsb[:, 4096:8192])

            # ---------------- software-pipelined emission ----------------
            if stages <= 1:
                sbms = emit_loads(0)
                for i in range(NCH):
                    nxt_sb = emit_loads(i + 1) if i + 1 < NCH else None
                    xT = emit_in_transposes(sbms)
                    dmp = out.bitcast(bf16).rearrange(
                        "(c p g) f -> c p (g f)", c=NCH, p=128, g=4)[i]
                    nc.scalar.dma_start(out=dmp[:, :8192], in_=xT)
                    sbms = nxt_sb
            else:
                sbms0 = emit_loads(0)
                sbms1 = emit_loads(1)
                xT_cur = emit_in_transposes(sbms0)
                z1_prev = None
                for ch in range(NCH):
                    nxt = None
                    if ch + 1 < NCH:
                        nxt = (alloc_xT(), sbms1)
                    pload = ((lambda c=ch: emit_loads(c + 2))
                             if ch + 2 < NCH else None)
                    z1_cur, pl = emit_l0(ch, xT_cur, nxt, postload=pload)
                    if pl is not None:
                        sbms1 = pl
                    if z1_prev is not None:
                        emit_l1(ch - 1, z1_prev)
                    z1_prev = z1_cur
                    if nxt is not None:
                        xT_cur = nxt[0]
                if z1_prev is not None:
                    emit_l1(NCH - 1, z1_prev)

    nc.compile()
    return nc


def _get_module():
    if "m" not in _module_cache:
        _module_cache["m"] = build(act="gelu")
    return _module_cache["m"]


def kernel(**inputs):
    import ml_dtypes
    from concourse import bass_utils
    nc = _get_module()
    bf = ml_dtypes.bfloat16
    x = np.ascontiguousarray(np.asarray(inputs["x"]).astype(bf))
    names = ["W1_0", "W1_1", "W2_0", "W2_1"]
    wmap = {k: np.ascontiguousarray(np.asarray(inputs[k]).astype(bf))
            for k in names}
    in_maps = []
    for c in range(NCORES):
        m = dict(wmap)
        m["x"] = np.ascontiguousarray(x[c * BCORE:(c + 1) * BCORE])
        in_maps.append(m)
    res = bass_utils.run_bass_kernel_spmd(nc, in_maps, core_ids=list(range(NCORES)))
    return np.concatenate([res.results[c]["out"] for c in range(NCORES)], axis=0)


# revision 23
# speedup vs baseline: 1.0424x; 1.0340x over previous
"""Trainium2 Bass kernel: nn_BlockMLP_MixerBlock (2-layer butterfly block-MLP mixer).

Math (per batch row pair; BS=16384, D=2048, BD=64, NB=32, H=128):
  L0: per block n: o = gelu(y @ W1_0[n]) @ W2_0[n]   (biases are zeros by spec)
  P1 butterfly: element (b=2q+r, block n, pos j=32u+v) -> (b'=2q+u, block v, pos 32r+n)
  L1: same block-MLP with W*_1;  P2 = same involution.

v5: all-bf16 PE; software-pipelined chunk emission (input stage of chunk i
interleaves with the body of chunk i-1); copies are blits or low-dim APs; the
matmul-rhs single-free-dim rule is satisfied with single *strided* dims
(z1 read at stride 32, out-transpose reads at stride 2).

Batch labels within a chunk of 512 rows (256 pairs): pair q_local =
128*pt + 64*q0 + ph (pt = load tile, q0 = sbm partition hi-bit, ph = low 6),
parity r.  qqb := pt.  Layouts per chunk (bf16 except outsb):
  xT    [128 p=f%128,  free (t 16, q0 2, pt 2, r 2, ph 64)]
  hsb   [128 p=H,      free (s 2, q0 2, pt 2, r 2, ph 64)]     gelu blit
  o0sb  [128 p=(q0,32u+v), free (ph 64, pt 2, r 2, t 16, s 2)]
  z1sb  [128 p=(qqb,32r+n), free (g 8, k 8, q0 2, u 2, v 32)]  blit of psM
  o1sb  [128 p=(qqb,32r'+n'), free (v 32, gk 64, q0 2, u 2)]   blit of o1ps
  outsb [128 p=(2ph+q0), free (qqb 2, r' 2, n' 32, u 2, v 32)] f32
"""
import numpy as np

BS, D, BD, NB, H = 16384, 2048, 64, 32, 128
NCORES = 8
BCORE = BS // NCORES     # 2048
CB = 512                 # chunk rows
NCH = BCORE // CB        # 4

_module_cache = {}


def build(act="gelu", stages=5):
    import concourse.mybir as mybir
    from concourse import bacc
    from concourse.tile import TileContext
    from concourse.masks import make_identity

    f32 = mybir.dt.float32
    bf16 = mybir.dt.bfloat16
    AF = mybir.ActivationFunctionType
    act_fn = AF.Gelu if act == "gelu" else AF.Copy

    nc = bacc.Bacc("TRN2", target_bir_lowering=False)
    x = nc.dram_tensor("x", (BCORE, D), bf16, kind="ExternalInput")
    W1d = [nc.dram_tensor("W1_0", (NB, BD, H), bf16, kind="ExternalInput"),
           nc.dram_tensor("W1_1", (NB, BD, H), bf16, kind="ExternalInput")]
    W2d = [nc.dram_tensor("W2_0", (NB, H, BD), bf16, kind="ExternalInput"),
           nc.dram_tensor("W2_1", (NB, H, BD), bf16, kind="ExternalInput")]
    out = nc.dram_tensor("out", (BCORE, D), f32, kind="ExternalOutput")

    with TileContext(nc) as tc:
        with tc.tile_pool(name="wp", bufs=1) as wp, \
             tc.tile_pool(name="sbp", bufs=4) as sbp, \
             tc.tile_pool(name="xtp", bufs=2) as xtp, \
             tc.tile_pool(name="o0p", bufs=2) as o0p, \
             tc.tile_pool(name="z1p", bufs=2) as z1p, \
             tc.tile_pool(name="o1p", bufs=1) as o1p, \
             tc.tile_pool(name="outp", bufs=1) as outp, \
             tc.tile_pool(name="wk", bufs=3) as wk, \
             tc.tile_pool(name="pst", bufs=2, space="PSUM") as pst, \
             tc.tile_pool(name="pss", bufs=2, space="PSUM") as pss, \
             tc.tile_pool(name="psh", bufs=2, space="PSUM") as psh:

            # ---------------- weights (staged once; scalar queue) ----------
            ident = wp.tile([128, 128], bf16, name="ident", tag="ident")
            make_identity(nc, ident)

            # L0 MM1: lhsT for block n=2t+s at partitions [64s,64s+64)
            w1l0 = wp.tile([128, 16 * 128], bf16, name="w1l0", tag="w1l0")
            w1r0 = W1d[0].rearrange("(t s) c m -> s c t m", s=2)
            for s in range(2):
                nc.scalar.dma_start(out=w1l0[64 * s:64 * s + 64, :], in_=w1r0[s])
            # L1 MM1: W1_1 duplicated in both partition halves; used with
            # tile_position=(64*qqb, 0) and K=64 rhs slices.
            w1l1 = wp.tile([128, 32 * 128], bf16, name="w1l1", tag="w1l1")
            w1r1 = W1d[1].rearrange("v c m -> c v m")
            for h2 in range(2):
                nc.scalar.dma_start(out=w1l1[64 * h2:64 * h2 + 64, :], in_=w1r1)
            # MM2 (both layers): interleaved [Z | W2_l | Z | ... | Z]
            # A(n) = [:, 128n+64:+128] = [W2_n | Z], B(n) = [:, 128n:+128]
            w2l = []
            for l in range(2):
                w2t = wp.tile([128, 64 * (2 * NB + 1)], bf16,
                              name=f"w2l{l}", tag=f"w2l{l}")
                zv = w2t.rearrange("p (m c) -> p m c", c=64)[:, ::2]
                nc.gpsimd.memset(zv, 0.0)
                dst = w2t[:, 0:4096].rearrange("p (n c) -> p n c", c=128)[:, :, 64:128]
                nc.scalar.dma_start(out=dst,
                                    in_=W2d[l].rearrange("n m j -> m n j"))
                w2l.append(w2t)

            xv = x.rearrange("(q r) f -> q r f", r=2)   # q global pair, r parity

            def emit_loads(ch):
                sbms = {}
                for r in range(2):
                    for pt in range(2):
                        sbm = sbp.tile([128, D], bf16, name=f"sbm{r}{pt}",
                                       tag="sbm")
                        q0g = ch * 256 + 128 * pt
                        nc.sync.dma_start(out=sbm, in_=xv[q0g:q0g + 128, r, :])
                        sbms[(r, pt)] = sbm
                return sbms

            IN_GROUPS = [(g8, r, pt) for g8 in range(2)
                         for r in range(2) for pt in range(2)]

            def alloc_xT():
                # xT col (within t) = q0*256 + pt*128 + r*64 + ph
                return xtp.tile([128, 8192], bf16, name="xT", tag="xT")

            def emit_in_group(xT, sbms, gi):
                g8, r, pt = IN_GROUPS[gi]
                xTr = xT.rearrange("p (t q0x pt2 r2 ph) -> r2 pt2 p t q0x ph",
                                   t=16, q0x=2, pt2=2, r2=2, ph=64)
                sbm = sbms[(r, pt)]
                psT = pst.tile([128, 1024], bf16, name="psT", tag="tr")
                for k in range(8):
                    ft = 8 * g8 + k
                    nc.tensor.transpose(
                        out=psT[:, 128 * k:128 * k + 128],
                        in_=sbm[:, 128 * ft:128 * ft + 128],
                        identity=ident)
                nc.vector.tensor_copy(
                    out=xTr[r, pt][:, 8 * g8:8 * g8 + 8],
                    in_=psT.rearrange("p (k q0x ph) -> p k q0x ph",
                                      k=8, q0x=2, ph=64))

            def emit_in_transposes(sbms):
                xT = alloc_xT()
                for gi in range(8):
                    emit_in_group(xT, sbms, gi)
                return xT

            def emit_body(ch, xT, nxt=None):
                # ---------------- layer 0 ----------------
                o0sb = o0p.tile([128, 8192], bf16, name="o0sb", tag="o0")
                o0m = o0sb.rearrange("p (ph ptr nt s) -> nt p ptr ph s",
                                     ph=64, ptr=4, nt=16, s=2)

                def l0_mm1(t):
                    hps = psh.tile([128, 1024], f32, name="hps", tag="h")
                    for s in range(2):
                        nc.tensor.matmul(
                            hps[:, 512 * s:512 * s + 512],
                            w1l0[64 * s:64 * s + 64, 128 * t:128 * t + 128],
                            xT[64 * s:64 * s + 64, 512 * t:512 * t + 512],
                            start=True, stop=True, tile_position=(64 * s, 0))
                    hsb = wk.tile([128, 1024], bf16, name="hsb", tag="hsb")
                    nc.scalar.activation(hsb, hps, act_fn)
                    return hsb

                # MM1(t+1) is emitted before MM2(t): the in-order PE queue
                # then streams matmuls while ACT computes gelu(t).
                hq = [l0_mm1(0), l0_mm1(1)]
                for t in range(16):
                    if t + 2 < 16:
                        hq.append(l0_mm1(t + 2))
                    hsb = hq.pop(0)
                    ops = pss.tile([128, 512], f32, name="ops", tag="mm")
                    for s in range(2):
                        n = 2 * t + s
                        # zero-pad accumulate: out partitions (q0, j=32u+v)
                        nc.tensor.matmul(
                            ops[:, 256 * s:256 * s + 256],
                            w2l[0][:, 128 * n + 64:128 * n + 192],
                            hsb[:, 512 * s:512 * s + 256],
                            start=True, stop=False)
                        nc.tensor.matmul(
                            ops[:, 256 * s:256 * s + 256],
                            w2l[0][:, 128 * n:128 * n + 128],
                            hsb[:, 512 * s + 256:512 * s + 512],
                            start=False, stop=True)
                    # one 3D copy per t: dims (ptr, ph, s); dst s-stride 1
                    opr = ops.rearrange("p (s ptr ph) -> p ptr ph s",
                                        s=2, ptr=4, ph=64)
                    nc.vector.tensor_copy(out=o0m[t], in_=opr)
                    # interleave next chunk's input transposes into L0
                    if nxt is not None and t % 2 == 1:
                        emit_in_group(nxt[0], nxt[1], t // 2)

                if stages <= 2:
                    dmp = out.bitcast(bf16).rearrange(
                        "(c p g) f -> c p (g f)", c=NCH, p=128, g=4)[ch]
                    nc.scalar.dma_start(out=dmp[:, :8192], in_=o0sb)
                    return
                # ---------------- mid transposes (P1) ----------------
                # o0sb slice [128ph:+128] = (pt, r, n=2t+s) -> partitions
                # (qqb=pt, rn);  z1sb = straight blit of psM.
                z1sb = z1p.tile([128, 8192], bf16, name="z1sb", tag="z1")
                for g in range(8):
                    psM = pst.tile([128, 1024], bf16, name="psM", tag="tr")
                    for k in range(8):
                        ph = 8 * g + k
                        nc.tensor.transpose(
                            out=psM[:, 128 * k:128 * k + 128],
                            in_=o0sb[:, 128 * ph:128 * ph + 128],
                            identity=ident)
                    nc.vector.tensor_copy(
                        out=z1sb[:, 1024 * g:1024 * g + 1024], in_=psM)

                if stages <= 3:
                    dmp = out.bitcast(bf16).rearrange(
                        "(c p g) f -> c p (g f)", c=NCH, p=128, g=4)[ch]
                    nc.scalar.dma_start(out=dmp[:, :8192], in_=z1sb)
                    return
                # ---------------- layer 1 ----------------
                # z1 col = 32*j + v with j = (g k q0 u): rhs per v is one
                # strided free dim (stride 32).  h1/o1 cols iterate j.
                z1v = z1sb.rearrange("p (j v) -> v p j", j=256, v=32)
                o1sb = o1p.tile([128, 8192], bf16, name="o1sb", tag="o1")

                def l1_mm1(G):
                    # h1ps cols = (qqb, w, j): tile_position pair must hit
                    # different PSUM banks (concurrent quadrant matmuls).
                    h1ps = psh.tile([128, 1024], f32, name="h1ps", tag="h")
                    for w in range(2):
                        v = 2 * G + w
                        for qqb in range(2):
                            nc.tensor.matmul(
                                h1ps[:, 512 * qqb + 256 * w:512 * qqb + 256 * w + 256],
                                w1l1[64 * qqb:64 * qqb + 64, 128 * v:128 * v + 128],
                                z1v[v][64 * qqb:64 * qqb + 64],
                                start=True, stop=True,
                                tile_position=(64 * qqb, 0))
                    h1sb = wk.tile([128, 1024], bf16, name="h1sb", tag="h1sb")
                    nc.scalar.activation(h1sb, h1ps, act_fn)
                    return h1sb

                # out-transpose plumbing (interleaved into the L1 loop:
                # group (u, G4) unlocks after o1 blits 4*G4..4*G4+3)
                o1r = o1sb.rearrange("p (v j2 u) -> v u p j2",
                                     v=32, j2=128, u=2)
                outsb = outp.tile([128, 8192], f32, name="outsb", tag="outsb")
                outr = outsb.rearrange(
                    "p (qqb jhi jlo uu gg k) -> gg uu qqb p k jhi jlo",
                    qqb=2, jhi=2, jlo=32, uu=2, gg=4, k=8)

                def emit_outT(G4):
                    for u in range(2):
                        psO = pst.tile([128, 1024], bf16, name="psO", tag="tr")
                        for k in range(8):
                            v = 8 * G4 + k
                            nc.tensor.transpose(
                                out=psO[:, 128 * k:128 * k + 128],
                                in_=o1r[v, u],
                                identity=ident)
                        psr = psO.rearrange(
                            "p (k qqb jhi jlo) -> qqb p k jhi jlo",
                            k=8, qqb=2, jhi=2, jlo=32)
                        nc.vector.tensor_copy(out=outr[G4, u, 0], in_=psr[0])
                        nc.scalar.activation(outr[G4, u, 1], psr[1], AF.Copy)

                h1q = [l1_mm1(0), l1_mm1(1)]
                for G in range(16):  # 2 blocks per group
                    if G + 2 < 16:
                        h1q.append(l1_mm1(G + 2))
                    h1sb = h1q.pop(0)
                    o1ps = pss.tile([128, 512], f32, name="o1ps", tag="mm")
                    for w in range(2):
                        v = 2 * G + w
                        # zero-pad accumulate: out partitions (qqb, 32r'+n')
                        nc.tensor.matmul(
                            o1ps[:, 256 * w:256 * w + 256],
                            w2l[1][:, 128 * v + 64:128 * v + 192],
                            h1sb[:, 256 * w:256 * w + 256],
                            start=True, stop=False)
                        nc.tensor.matmul(
                            o1ps[:, 256 * w:256 * w + 256],
                            w2l[1][:, 128 * v:128 * v + 128],
                            h1sb[:, 512 + 256 * w:512 + 256 * w + 256],
                            start=False, stop=True)
                    nc.vector.tensor_copy(
                        out=o1sb[:, 512 * G:512 * G + 512], in_=o1ps)
                    if stages > 4 and G % 4 == 3:
                        emit_outT(G // 4)

                if stages <= 4:
                    dmp = out.bitcast(bf16).rearrange(
                        "(c p g) f -> c p (g f)", c=NCH, p=128, g=4)[ch]
                    nc.scalar.dma_start(out=dmp[:, :8192], in_=o1sb)
                    return
                # ---------------- store (scalar queue) ----------------
                # DRAM row = ch*512 + qqb*256 + q0*128 + ph*2 + r_out
                ov = out.rearrange("(c qx q0x pp jhi) f -> c qx pp q0x (jhi f)",
                                   c=NCH, qx=2, q0x=2, pp=64, jhi=2)
                nc.scalar.dma_start(out=ov[ch, 0], in_=outsb[:, 0:4096])
                eng_st = nc.sync if ch == NCH - 1 else nc.scalar
                eng_st.dma_start(out=ov[ch, 1], in_=outsb[:, 4096:8192])

            # ---------------- software-pipelined emission ----------------
            if stages <= 1:
                sbms = emit_loads(0)
                for i in range(NCH):
                    nxt_sb = emit_loads(i + 1) if i + 1 < NCH else None
                    xT = emit_in_transposes(sbms)
                    dmp = out.bitcast(bf16).rearrange(
                        "(c p g) f -> c p (g f)", c=NCH, p=128, g=4)[i]
                    nc.scalar.dma_start(out=dmp[:, :8192], in_=xT)
                    sbms = nxt_sb
            else:
                sbms0 = emit_loads(0)
                sbms1 = emit_loads(1)
                xT_cur = emit_in_transposes(sbms0)
                for ch in range(NCH):
                    nxt = None
                    if ch + 1 < NCH:
                        nxt = (alloc_xT(), sbms1)
                    emit_body(ch, xT_cur, nxt)
                    if ch + 2 < NCH:
                        sbms1 = emit_loads(ch + 2)
                    if nxt is not None:
                        xT_cur = nxt[0]

    nc.compile()
    return nc


def _get_module():
    if "m" not in _module_cache:
        _module_cache["m"] = build(act="gelu")
    return _module_cache["m"]


def kernel(**inputs):
    import ml_dtypes
    from concourse import bass_utils
    nc = _get_module()
    bf = ml_dtypes.bfloat16
    x = np.ascontiguousarray(np.asarray(inputs["x"]).astype(bf))
    names = ["W1_0", "W1_1", "W2_0", "W2_1"]
    wmap = {k: np.ascontiguousarray(np.asarray(inputs[k]).astype(bf))
            for k in names}
    in_maps = []
    for c in range(NCORES):
        m = dict(wmap)
        m["x"] = np.ascontiguousarray(x[c * BCORE:(c + 1) * BCORE])
        in_maps.append(m)
    res = bass_utils.run_bass_kernel_spmd(nc, in_maps, core_ids=list(range(NCORES)))
    return np.concatenate([res.results[c]["out"] for c in range(NCORES)], axis=0)
